# revision 15
# baseline (speedup 1.0000x reference)
"""Trainium2 Bass kernel for nn_CompressedCausalAttention.

Sharding: 8 cores = 2 batches x 4 head-groups (2 heads each).
Host prep: xpe = (x+pe)^T in bf16 (cuts HBM reads ~3x and removes the
on-device add); weights transposed/sliced per core.

Per-core dataflow — one flat software-pipelined loop over all 40
(window, t-chunk) attention chunks:
  sc(c+1) | exp(c) | filler | av(c)
so ACT (exp) runs back-to-back — it is the bottleneck engine — while PE
fills its idle slots with interleaved work: next window's qkv
projections, the previous window's softmax tail (reciprocal-normalize)
and partial out-projection.  Per-head score matmuls are K=64 and land on
disjoint PE row-groups (partitions 0-63 / 64-127), so each pair runs
concurrently on the array.
  qkv:  q,k chan-major (bias via DVE tensor_scalar_add); v seq-major,
        staged to SBUF by DVE with a ones column (softmax denominator
        rides the AV matmul).
  attn: scores in (t-part, s-free), staircase mask via PE tri-matmul,
        exp on ACT, AV accumulated in PSUM.
  tail: den row -> SBUF copy -> reciprocal_approx_fast (DVE; reading
        PSUM directly returns garbage on HW) -> gpsimd partition
        broadcast -> DVE multiply.
  out:  partial out-projection, ob copy to bf16 (DVE; split with ACT in
        the epilogue), DMA out issued from the gpsimd queue.
Weight DMAs issue from the gpsimd queue, xpe loads from sync, so the
prologue is not serialized on one issue queue.
Host: sums the 4 per-batch partials, adds bc (+ v-bias folded through Wc).
"""

import numpy as np
import ml_dtypes

S, B, C, H = 2048, 2, 512, 8
CC = C // H            # 64
HPC = 2                # heads per core
NCORE = 8
SW = 512               # s window (free dim of score tiles)
TCH = 128              # t chunk (partition dim of score tiles)
NW = S // SW           # 4 windows
TEMP = 1.0 / 8.0       # 1/sqrt(CC)
BIGNEG = -30000.0

_CACHE = {}


def _build_bass():
    import concourse.bass as bass
    import concourse.mybir as mybir
    import concourse.tile as tile
    from concourse import bacc

    f32 = mybir.dt.float32
    bf16 = mybir.dt.bfloat16

    nc = bacc.Bacc("TRN2", target_bir_lowering=False)
    xpet = nc.declare_dram_parameter("xpet", [NW, 4, 128, SW], bf16, isOutput=False)
    w3t = nc.declare_dram_parameter("w3t", [4, 128, 384], bf16, isOutput=False)
    b3 = nc.declare_dram_parameter("b3", [128, 2], f32, isOutput=False)
    wct = nc.declare_dram_parameter("wct", [128, C], bf16, isOutput=False)
    tri = nc.declare_dram_parameter("tri", [128, 128], bf16, isOutput=False)
    outp = nc.declare_dram_parameter("outp", [C, S], bf16, isOutput=True)

    Exp = mybir.ActivationFunctionType.Exp

    chunks = [(i, j) for i in range(NW) for j in range(4 * i + 4)]
    NCH = len(chunks)   # 40

    with tile.TileContext(nc) as tc:
        with (
            tc.tile_pool(name="singles", bufs=1) as singles,
            tc.tile_pool(name="xp", bufs=4) as xp,
            tc.tile_pool(name="pbp", bufs=4) as pbp,
            tc.tile_pool(name="atp", bufs=2) as atp,
            tc.tile_pool(name="rbp", bufs=2) as rbp,
            tc.tile_pool(name="osp", bufs=4) as osp,
            tc.tile_pool(name="scp", bufs=2, space="PSUM") as scp,
            tc.tile_pool(name="avp", bufs=1, space="PSUM") as avp,
            tc.tile_pool(name="smp", bufs=2, space="PSUM") as smp,
        ):
            # ---- input DMAs fan out across all five engine issue queues so
            # the prologue is not serialized on one queue's ~600ns/issue ----
            w3t_sb = singles.tile([128, 4, 384], bf16, tag="w3t")
            xws = [xp.tile([128, 4, SW], bf16, tag="xpe", name=f"xw{w}")
                   for w in range(NW)]
            for k in range(4):
                nc.sync.dma_start(out=xws[0][:, k, 0:256], in_=xpet[0, k][:, 0:256])
                nc.scalar.dma_start(out=xws[0][:, k, 256:512],
                                    in_=xpet[0, k][:, 256:512])
                nc.gpsimd.dma_start(out=w3t_sb[:, k, :], in_=w3t[k])
            for k in range(4):
                nc.sync.dma_start(out=xws[1][:, k, 0:256], in_=xpet[1, k][:, 0:256])
                nc.scalar.dma_start(out=xws[1][:, k, 256:512],
                                    in_=xpet[1, k][:, 256:512])
            tri_sb = singles.tile([128, 128], bf16, tag="tri")
            nc.gpsimd.dma_start(out=tri_sb, in_=tri[:, :])
            b3_sb = singles.tile([128, 2], f32, tag="b3")
            nc.gpsimd.dma_start(out=b3_sb, in_=b3[:, :])
            wct_sb = singles.tile([128, C], bf16, tag="wct")
            nc.gpsimd.dma_start(out=wct_sb, in_=wct[:, :])
            for w in range(2, NW):
                for k in range(4):
                    nc.sync.dma_start(out=xws[w][:, k, :], in_=xpet[w, k])

            qTs = [singles.tile([128, SW], bf16, tag=f"qT{w}", name=f"qT{w}")
                   for w in range(NW)]
            kTs = [singles.tile([128, SW], bf16, tag=f"kT{w}", name=f"kT{w}")
                   for w in range(NW)]
            # v seq-major per window: [t-part, chunk, head, 64 ch + ones col]
            vsb = [singles.tile([128, 4, HPC, CC + 1], bf16, tag=f"vsb{w}",
                                name=f"vsb{w}") for w in range(NW)]
            for w in range(NW):
                nc.vector.memset(vsb[w][:, :, :, CC:CC + 1], 1.0)

            vps_store = {}

            qk_store = {}

            def qk_part(w, blk, half=None):
                dst = qTs[w] if blk == 0 else kTs[w]
                if half in (0, None):
                    qk_store[(w, blk)] = smp.tile([128, SW], f32, tag="sm",
                                                  name=f"qk{w}_{blk}")
                ps = qk_store[(w, blk)]
                ks = range(4) if half is None else ((0, 1) if half == 0 else (2, 3))
                for k in ks:
                    nc.tensor.matmul(
                        ps,
                        lhsT=w3t_sb[:, k, blk * 128:(blk + 1) * 128],
                        rhs=xws[w][:, k, :],
                        start=(k == 0), stop=(k == 3),
                    )
                if half in (1, None):
                    nc.vector.tensor_scalar_add(
                        out=dst, in0=ps, scalar1=b3_sb[:, blk:blk + 1],
                    )

            def v_part(w, tj):
                if tj == 0:
                    vps_store[w] = smp.tile([128, 4, HPC, CC], f32, tag="sm",
                                            name=f"vps{w}")
                vps = vps_store[w]
                for k in range(4):
                    nc.tensor.matmul(
                        vps[:, tj],
                        lhsT=xws[w][:, k, tj * TCH:(tj + 1) * TCH],
                        rhs=w3t_sb[:, k, 256:384],
                        start=(k == 0), stop=(k == 3),
                    )
                nc.vector.tensor_copy(out=vsb[w][:, tj, :, 0:CC],
                                      in_=vps[:, tj])

            avs_store = {}
            atn_store = {}

            def tail(i):
                avs, atn = avs_store[i], atn_store[i]
                r1s, rbs = [], []
                for h in range(HPC):
                    # den row staged to SBUF: reciprocal_approx_fast reads
                    # garbage from PSUM on HW
                    dd = rbp.tile([1, SW], f32, tag=f"dd{h}", name=f"dd{i}_{h}")
                    nc.vector.tensor_copy(out=dd, in_=avs[h][CC:CC + 1, :])
                    r1 = rbp.tile([1, SW], f32, tag=f"r1{h}", name=f"r1{i}_{h}")
                    nc.vector.reciprocal_approx_fast(out=r1, in_=dd)
                    r1s.append(r1)
                for h in range(HPC):
                    rb = rbp.tile([CC, SW], f32, tag=f"rb{h}", name=f"rb{i}_{h}")
                    nc.gpsimd.partition_broadcast(out_ap=rb, in_ap=r1s[h], channels=CC)
                    rbs.append(rb)
                for h in range(HPC):
                    nc.vector.tensor_mul(
                        out=atn[h * CC:(h + 1) * CC, :],
                        in0=avs[h][0:CC, :], in1=rbs[h],
                    )

            def outproj(i, ds, ob_engine=None):
                atn = atn_store[i]
                for d in ds:
                    op = smp.tile([128, SW], f32, tag="sm", name=f"op{i}_{d}")
                    nc.tensor.matmul(
                        op, lhsT=wct_sb[:, d * 128:(d + 1) * 128], rhs=atn,
                        start=True, stop=True,
                    )
                    ob = osp.tile([128, SW], bf16, tag="ob", name=f"ob{i}_{d}")
                    if ob_engine == "scalar":
                        nc.scalar.copy(out=ob, in_=op)
                    else:
                        nc.vector.tensor_copy(out=ob, in_=op)
                    nc.sync.dma_start(
                        out=outp[d * 128:(d + 1) * 128, i * SW:(i + 1) * SW],
                        in_=ob,
                    )

            # filler work items keyed by chunk index; window w starts at
            # chunk index 2*w*(w+1).  Items are sized ~0.5-1us of PE each so
            # a slot never starves the exp pipeline; deadlines: qk(w) before
            # the window-w scores, v(w,tj) before its AV chunk, tail(i)
            # pinned at the first chunk of window i+1 (avs psum reuse).
            fillers = {
                0: [lambda: v_part(0, 0)],
                1: [lambda: v_part(0, 1)],
                2: [lambda: v_part(0, 2)],
                3: [lambda: v_part(0, 3)],
                4: [lambda: tail(0)],
                5: [lambda: v_part(1, 0)],
                6: [lambda: v_part(1, 1)],
                7: [lambda: v_part(1, 2)],
                8: [lambda: qk_part(2, 0, 0)],
                9: [lambda: qk_part(2, 0, 1)],
                10: [lambda: qk_part(2, 1, 0), lambda: qk_part(2, 1, 1)],
                11: [lambda: v_part(1, 3)],
                12: [lambda: tail(1)],
                13: [lambda: qk_part(3, 0, 0)],
                14: [lambda: qk_part(3, 0, 1)],
                15: [lambda: qk_part(3, 1, 0)],
                16: [lambda: qk_part(3, 1, 1)],
                17: [lambda: v_part(2, 0)],
                18: [lambda: v_part(2, 1)],
                19: [lambda: v_part(2, 2)],
                20: [lambda: v_part(2, 3)],
                21: [lambda: outproj(0, (0, 1))],
                22: [lambda: outproj(0, (2, 3))],
                23: [lambda: outproj(1, (0, 1))],
                24: [lambda: tail(2)],
                25: [lambda: outproj(1, (2, 3))],
                26: [lambda: v_part(3, 0)],
                27: [lambda: v_part(3, 1)],
                28: [lambda: v_part(3, 2)],
                29: [lambda: v_part(3, 3)],
                32: [lambda: outproj(2, (0, 1))],
                33: [lambda: outproj(2, (2, 3))],
            }

            def emit_sc(c):
                i, j = chunks[c]
                if j == 0:
                    avs_store[i] = [
                        avp.tile([CC + 1, SW], f32, tag=f"av{h}", name=f"av{h}_{i}")
                        for h in range(HPC)
                    ]
                    atn_store[i] = atp.tile([128, SW], bf16, tag="atn",
                                            name=f"atn{i}")
                D = max(0, TCH * j - SW * i)
                wj, jj = j // 4, j % 4
                sc = scp.tile([128, HPC, SW], f32, tag="sc", name=f"sc{c}")
                for h in range(HPC):
                    nc.tensor.matmul(
                        sc[:, h, D:SW],
                        lhsT=kTs[wj][h * CC:(h + 1) * CC, jj * TCH:(jj + 1) * TCH],
                        rhs=qTs[i][h * CC:(h + 1) * CC, D:SW],
                        start=True, stop=True,
                    )
                return sc, D

            # ---- prologue: q/k(0) chunk-major (each matmul starts as soon
            # as its xpe chunk DMA lands), first scores, then q/k(1) ----
            def qk_chunk_major(w):
                qs = smp.tile([128, SW], f32, tag="sm", name=f"qk{w}_0")
                ks_ = smp.tile([128, SW], f32, tag="sm", name=f"qk{w}_1")
                qk_store[(w, 0)] = qs
                qk_store[(w, 1)] = ks_
                for k in range(4):
                    for blk, ps in ((0, qs), (1, ks_)):
                        nc.tensor.matmul(
                            ps,
                            lhsT=w3t_sb[:, k, blk * 128:(blk + 1) * 128],
                            rhs=xws[w][:, k, :],
                            start=(k == 0), stop=(k == 3),
                        )
                nc.vector.tensor_scalar_add(
                    out=qTs[w], in0=qs, scalar1=b3_sb[:, 0:1])
                nc.vector.tensor_scalar_add(
                    out=kTs[w], in0=ks_, scalar1=b3_sb[:, 1:2])

            # PE warm-up: ~16 junk matmuls on zeroed tiles fill the initial
            # DMA-latency window and bring HAM out of the half-clock state
            # before real work arrives
            wu_l = singles.tile([128, 128], bf16, tag="wu_l")
            nc.vector.memset(wu_l, 0.0)
            wu_r = singles.tile([128, SW], bf16, tag="wu_r")
            nc.vector.memset(wu_r, 0.0)
            wu_ps = smp.tile([128, SW], f32, tag="sm", name="wu_ps")
            for _ in range(16):
                nc.tensor.matmul(wu_ps, lhsT=wu_l, rhs=wu_r,
                                 start=True, stop=True, skip_group_check=True)

            from collections import deque
            qk_chunk_major(0)
            sc_q = deque([emit_sc(0), emit_sc(1)])
            qk_chunk_major(1)
            for c in range(NCH):
                i, j = chunks[c]
                sc, D = sc_q.popleft()
                pb = pbp.tile([128, HPC, SW], bf16, tag="pb", name=f"pb{c}")
                nc.scalar.activation(
                    out=pb[:, :, D:SW], in_=sc[:, :, D:SW], func=Exp, scale=TEMP,
                )
                if j >= 4 * i:
                    # causal mask on the diagonal block: zero the strictly-
                    # future probabilities (gpsimd, off the PE/ACT path)
                    for h in range(HPC):
                        nc.gpsimd.tensor_mul(
                            out=pb[:, h, D:D + TCH], in0=pb[:, h, D:D + TCH],
                            in1=tri_sb,
                        )
                for work in fillers.get(c, ()):
                    work()
                if c + 2 < NCH:
                    sc_q.append(emit_sc(c + 2))
                jmax = 4 * i + 3
                wj, jj = j // 4, j % 4
                for h in range(HPC):
                    nc.tensor.matmul(
                        avs_store[i][h][:, D:SW], lhsT=vsb[wj][:, jj, h, :],
                        rhs=pb[:, h, D:SW],
                        start=(j == 0), stop=(j == jmax),
                    )

            # ---- epilogue: window 3 tail + out-projection ----
            tail(NW - 1)
            outproj(NW - 1, (0, 2), ob_engine="scalar")
            outproj(NW - 1, (1, 3))

    nc.compile()
    return nc


def _get_nc():
    if "nc" not in _CACHE:
        _CACHE["nc"] = _build_bass()
    return _CACHE["nc"]


def _make_in_maps(x, pe, Wqkv, bqkv, Wc):
    bf = ml_dtypes.bfloat16
    tt = np.arange(128)[:, None]
    kk = np.arange(128)[None, :]
    tri = np.where(kk < tt, np.float32(0.0), np.float32(1.0)).astype(bf)

    xpet_b = {}
    for b in range(B):
        t = (x[:, b, :] + pe[:, b, :]).T.astype(bf).reshape(4, 128, NW, SW)
        xpet_b[b] = np.ascontiguousarray(t.transpose(2, 0, 1, 3))

    in_maps = []
    for core in range(NCORE):
        b, hg = core // 4, core % 4
        lo = hg * 128
        W3 = np.concatenate([Wqkv[lo:lo + 128], Wqkv[C + lo:C + lo + 128],
                             Wqkv[2 * C + lo:2 * C + lo + 128]])
        w3t = np.ascontiguousarray(W3.T).reshape(4, 128, 384).astype(bf)
        b3 = np.stack([bqkv[lo:lo + 128], bqkv[C + lo:C + lo + 128]], axis=1)
        b3 = np.ascontiguousarray(b3).astype(np.float32)
        wct = np.ascontiguousarray(Wc[:, lo:lo + 128].T).astype(bf)
        in_maps.append({
            "xpet": xpet_b[b], "w3t": w3t, "b3": b3,
            "wct": wct, "tri": tri,
        })
    return in_maps


def _numpy_fallback(x, pe, content_mask, Wqkv, bqkv, Wc, bc):
    xpe = (x + pe).astype(np.float32)
    qkv = xpe.reshape(-1, C) @ Wqkv.T + bqkv
    qkv = qkv.reshape(S, B, 3 * C)
    q, k, v = np.split(qkv, 3, axis=-1)
    q = q.reshape(S, B, H, CC)
    k = k.reshape(S, B, H, CC)
    v = v.reshape(S, B, H, CC)
    out = np.empty((S, B, C), np.float32)
    for b in range(B):
        for h in range(H):
            sc = (q[:, b, h] @ k[:, b, h].T) * np.float32(TEMP)
            sc = np.where(content_mask[:, :, b], -np.inf, sc)
            sc = sc - sc.max(axis=1, keepdims=True)
            p = np.exp(sc)
            p /= p.sum(axis=1, keepdims=True)
            out[:, b, h * CC:(h + 1) * CC] = p @ v[:, b, h]
    return (out.reshape(-1, C) @ Wc.T + bc).reshape(S, B, C).astype(np.float32)


def kernel(x, pe, content_mask, pad, Wqkv, bqkv, Wc, bc):
    x = np.asarray(x, dtype=np.float32)
    pe = np.asarray(pe, dtype=np.float32)
    content_mask = np.asarray(content_mask)
    Wqkv = np.asarray(Wqkv, dtype=np.float32)
    bqkv = np.asarray(bqkv, dtype=np.float32)
    Wc = np.asarray(Wc, dtype=np.float32)
    bc = np.asarray(bc, dtype=np.float32)

    idx = np.arange(S)
    causal = idx[None, :] > idx[:, None]
    if not np.array_equal(content_mask, np.broadcast_to(causal[:, :, None], (S, S, B))):
        return _numpy_fallback(x, pe, content_mask, Wqkv, bqkv, Wc, bc)

    from concourse.bass_utils import run_bass_kernel_spmd

    nc = _get_nc()
    in_maps = _make_in_maps(x, pe, Wqkv, bqkv, Wc)
    res = run_bass_kernel_spmd(nc, in_maps, core_ids=list(range(NCORE)))
    out = np.empty((S, B, C), np.float32)
    bc_eff = bc + Wc @ bqkv[2 * C:3 * C]   # v-bias folded through the output proj
    for b in range(B):
        acc = res.results[b * 4]["outp"].astype(np.float32).copy()
        for g in range(1, 4):
            acc += res.results[b * 4 + g]["outp"]
        out[:, b, :] = acc.T + bc_eff
    return out


# revision 16
# speedup vs baseline: 1.5061x; 1.5061x over previous
"""Trainium2 Bass kernel for nn_CompressedCausalAttention.

Sharding: 8 cores = 2 batches x 4 head-groups (2 heads each).
Host prep: xpe = (x+pe)^T in bf16 (cuts HBM reads ~3x and removes the
on-device add); weights transposed/sliced per core.

Per-core dataflow — one flat software-pipelined loop over all 40
(window, t-chunk) attention chunks:
  sc(c+1) | exp(c) | filler | av(c)
so ACT (exp) runs back-to-back — it is the bottleneck engine — while PE
fills its idle slots with interleaved work: next window's qkv
projections, the previous window's softmax tail (reciprocal-normalize)
and partial out-projection.  Per-head score matmuls are K=64 and land on
disjoint PE row-groups (partitions 0-63 / 64-127), so each pair runs
concurrently on the array.
  qkv:  q,k chan-major (bias via DVE tensor_scalar_add); v seq-major,
        staged to SBUF by DVE with a ones column (softmax denominator
        rides the AV matmul).
  attn: scores in (t-part, s-free), staircase mask via PE tri-matmul,
        exp on ACT, AV accumulated in PSUM.
  tail: den row -> SBUF copy -> reciprocal_approx_fast (DVE; reading
        PSUM directly returns garbage on HW) -> gpsimd partition
        broadcast -> DVE multiply.
  out:  partial out-projection, ob copy to bf16 (DVE; split with ACT in
        the epilogue), DMA out issued from the gpsimd queue.
Weight DMAs issue from the gpsimd queue, xpe loads from sync, so the
prologue is not serialized on one issue queue.
Host: sums the 4 per-batch partials, adds bc (+ v-bias folded through Wc).
"""

import numpy as np
import ml_dtypes

S, B, C, H = 2048, 2, 512, 8
CC = C // H            # 64
HPC = 2                # heads per core
NCORE = 8
SW = 512               # s window (free dim of score tiles)
TCH = 128              # t chunk (partition dim of score tiles)
NW = S // SW           # 4 windows
TEMP = 1.0 / 8.0       # 1/sqrt(CC)
BIGNEG = -30000.0

_CACHE = {}


def _build_bass():
    import concourse.bass as bass
    import concourse.mybir as mybir
    import concourse.tile as tile
    from concourse import bacc

    f32 = mybir.dt.float32
    bf16 = mybir.dt.bfloat16

    nc = bacc.Bacc("TRN2", target_bir_lowering=False)
    xpet = nc.declare_dram_parameter("xpet", [NW, 4, 128, SW], bf16, isOutput=False)
    w3t = nc.declare_dram_parameter("w3t", [4, 128, 384], bf16, isOutput=False)
    b3 = nc.declare_dram_parameter("b3", [128, 2], f32, isOutput=False)
    wct = nc.declare_dram_parameter("wct", [128, C], bf16, isOutput=False)
    tri = nc.declare_dram_parameter("tri", [128, 128], bf16, isOutput=False)
    id128 = nc.declare_dram_parameter("id128", [128, 128], bf16, isOutput=False)
    outp = nc.declare_dram_parameter("outp", [C, S], bf16, isOutput=True)

    Exp = mybir.ActivationFunctionType.Exp

    chunks = [(i, j) for i in range(NW) for j in range(4 * i + 4)]
    NCH = len(chunks)   # 40

    with tile.TileContext(nc) as tc:
        with (
            tc.tile_pool(name="singles", bufs=1) as singles,
            tc.tile_pool(name="xp", bufs=4) as xp,
            tc.tile_pool(name="pbp", bufs=4) as pbp,
            tc.tile_pool(name="atp", bufs=2) as atp,
            tc.tile_pool(name="rbp", bufs=2) as rbp,
            tc.tile_pool(name="osp", bufs=4) as osp,
            tc.tile_pool(name="scp", bufs=2, space="PSUM") as scp,
            tc.tile_pool(name="avp", bufs=1, space="PSUM") as avp,
            tc.tile_pool(name="smp", bufs=2, space="PSUM") as smp,
        ):
            # ---- input DMAs fan out across all five engine issue queues so
            # the prologue is not serialized on one queue's ~600ns/issue ----
            w3t_sb = singles.tile([128, 4, 384], bf16, tag="w3t")
            xws = [xp.tile([128, 4, SW], bf16, tag="xpe", name=f"xw{w}")
                   for w in range(NW)]
            for k in range(4):
                nc.sync.dma_start(out=xws[0][:, k, 0:256], in_=xpet[0, k][:, 0:256])
                nc.scalar.dma_start(out=xws[0][:, k, 256:512],
                                    in_=xpet[0, k][:, 256:512])
                nc.gpsimd.dma_start(out=w3t_sb[:, k, :], in_=w3t[k])
            for k in range(4):
                nc.sync.dma_start(out=xws[1][:, k, 0:256], in_=xpet[1, k][:, 0:256])
                nc.scalar.dma_start(out=xws[1][:, k, 256:512],
                                    in_=xpet[1, k][:, 256:512])
            tri_sb = singles.tile([128, 128], bf16, tag="tri")
            nc.gpsimd.dma_start(out=tri_sb, in_=tri[:, :])
            id_sb = singles.tile([128, 128], bf16, tag="id128")
            nc.gpsimd.dma_start(out=id_sb, in_=id128[:, :])
            b3_sb = singles.tile([128, 2], f32, tag="b3")
            nc.gpsimd.dma_start(out=b3_sb, in_=b3[:, :])
            wct_sb = singles.tile([128, C], bf16, tag="wct")
            nc.gpsimd.dma_start(out=wct_sb, in_=wct[:, :])
            for w in range(2, NW):
                for k in range(4):
                    nc.sync.dma_start(out=xws[w][:, k, :], in_=xpet[w, k])

            qTs = [singles.tile([128, SW], bf16, tag=f"qT{w}", name=f"qT{w}")
                   for w in range(NW)]
            kTs = [singles.tile([128, SW], bf16, tag=f"kT{w}", name=f"kT{w}")
                   for w in range(NW)]
            # v seq-major per window: [t-part, chunk, head, 64 ch + ones col]
            vsb = [singles.tile([128, 4, HPC, CC + 1], bf16, tag=f"vsb{w}",
                                name=f"vsb{w}") for w in range(NW)]
            for w in range(NW):
                nc.vector.memset(vsb[w][:, :, :, CC:CC + 1], 1.0)

            vps_store = {}

            qk_store = {}

            def qk_part(w, blk, half=None):
                dst = qTs[w] if blk == 0 else kTs[w]
                if half in (0, None):
                    qk_store[(w, blk)] = smp.tile([128, SW], f32, tag="sm",
                                                  name=f"qk{w}_{blk}")
                ps = qk_store[(w, blk)]
                ks = range(4) if half is None else ((0, 1) if half == 0 else (2, 3))
                for k in ks:
                    nc.tensor.matmul(
                        ps,
                        lhsT=w3t_sb[:, k, blk * 128:(blk + 1) * 128],
                        rhs=xws[w][:, k, :],
                        start=(k == 0), stop=(k == 3),
                    )
                if half in (1, None):
                    nc.vector.tensor_scalar_add(
                        out=dst, in0=ps, scalar1=b3_sb[:, blk:blk + 1],
                    )

            def v_part(w, tj):
                if tj == 0:
                    vps_store[w] = smp.tile([128, 4, HPC, CC], f32, tag="sm",
                                            name=f"vps{w}")
                vps = vps_store[w]
                for k in range(4):
                    nc.tensor.matmul(
                        vps[:, tj],
                        lhsT=xws[w][:, k, tj * TCH:(tj + 1) * TCH],
                        rhs=w3t_sb[:, k, 256:384],
                        start=(k == 0), stop=(k == 3),
                    )
                nc.vector.tensor_copy(out=vsb[w][:, tj, :, 0:CC],
                                      in_=vps[:, tj])

            avs_store = {}
            atn_store = {}

            def tail(i):
                avs, atn = avs_store[i], atn_store[i]
                r1s, rbs = [], []
                for h in range(HPC):
                    # den row staged to SBUF: reciprocal_approx_fast reads
                    # garbage from PSUM on HW
                    dd = rbp.tile([1, SW], f32, tag=f"dd{h}", name=f"dd{i}_{h}")
                    nc.vector.tensor_copy(out=dd, in_=avs[h][CC:CC + 1, :])
                    r1 = rbp.tile([1, SW], f32, tag=f"r1{h}", name=f"r1{i}_{h}")
                    nc.vector.reciprocal_approx_fast(out=r1, in_=dd)
                    r1s.append(r1)
                for h in range(HPC):
                    rb = rbp.tile([CC, SW], f32, tag=f"rb{h}", name=f"rb{i}_{h}")
                    nc.gpsimd.partition_broadcast(out_ap=rb, in_ap=r1s[h], channels=CC)
                    rbs.append(rb)
                for h in range(HPC):
                    nc.vector.tensor_mul(
                        out=atn[h * CC:(h + 1) * CC, :],
                        in0=avs[h][0:CC, :], in1=rbs[h],
                    )

            def outproj(i, ds, ob_engine=None):
                atn = atn_store[i]
                for d in ds:
                    op = smp.tile([128, SW], f32, tag="sm", name=f"op{i}_{d}")
                    nc.tensor.matmul(
                        op, lhsT=wct_sb[:, d * 128:(d + 1) * 128], rhs=atn,
                        start=True, stop=True,
                    )
                    ob = osp.tile([128, SW], bf16, tag="ob", name=f"ob{i}_{d}")
                    if ob_engine == "scalar":
                        nc.scalar.copy(out=ob, in_=op)
                    else:
                        nc.vector.tensor_copy(out=ob, in_=op)
                    nc.sync.dma_start(
                        out=outp[d * 128:(d + 1) * 128, i * SW:(i + 1) * SW],
                        in_=ob,
                    )

            # filler work items keyed by chunk index; window w starts at
            # chunk index 2*w*(w+1).  Items are sized ~0.5-1us of PE each so
            # a slot never starves the exp pipeline; deadlines: qk(w) before
            # the window-w scores, v(w,tj) before its AV chunk, tail(i)
            # pinned at the first chunk of window i+1 (avs psum reuse).
            fillers = {
                0: [lambda: v_part(0, 0)],
                1: [lambda: v_part(0, 1)],
                2: [lambda: v_part(0, 2)],
                3: [lambda: v_part(0, 3)],
                4: [lambda: tail(0)],
                5: [lambda: v_part(1, 0)],
                6: [lambda: v_part(1, 1)],
                7: [lambda: v_part(1, 2)],
                8: [lambda: qk_part(2, 0, 0)],
                9: [lambda: qk_part(2, 0, 1)],
                10: [lambda: qk_part(2, 1, 0), lambda: qk_part(2, 1, 1)],
                11: [lambda: v_part(1, 3)],
                12: [lambda: tail(1)],
                13: [lambda: qk_part(3, 0, 0)],
                14: [lambda: qk_part(3, 0, 1)],
                15: [lambda: qk_part(3, 1, 0)],
                16: [lambda: qk_part(3, 1, 1)],
                17: [lambda: v_part(2, 0)],
                18: [lambda: v_part(2, 1)],
                19: [lambda: v_part(2, 2)],
                20: [lambda: v_part(2, 3)],
                21: [lambda: outproj(0, (0, 1))],
                22: [lambda: outproj(0, (2, 3))],
                23: [lambda: outproj(1, (0, 1))],
                24: [lambda: tail(2)],
                25: [lambda: outproj(1, (2, 3))],
                26: [lambda: v_part(3, 0)],
                27: [lambda: v_part(3, 1)],
                28: [lambda: v_part(3, 2)],
                29: [lambda: v_part(3, 3)],
                32: [lambda: outproj(2, (0, 1))],
                33: [lambda: outproj(2, (2, 3))],
            }

            def emit_sc(c):
                i, j = chunks[c]
                if j == 0:
                    avs_store[i] = [
                        avp.tile([CC + 1, SW], f32, tag=f"av{h}", name=f"av{h}_{i}")
                        for h in range(HPC)
                    ]
                    atn_store[i] = atp.tile([128, SW], bf16, tag="atn",
                                            name=f"atn{i}")
                D = max(0, TCH * j - SW * i)
                wj, jj = j // 4, j % 4
                sc = scp.tile([128, HPC, SW], f32, tag="sc", name=f"sc{c}")
                for h in range(HPC):
                    nc.tensor.matmul(
                        sc[:, h, D:SW],
                        lhsT=kTs[wj][h * CC:(h + 1) * CC, jj * TCH:(jj + 1) * TCH],
                        rhs=qTs[i][h * CC:(h + 1) * CC, D:SW],
                        start=True, stop=True,
                    )
                if j >= 4 * i:
                    # staircase mask: accumulate -30000*lower_tri via PE
                    for h in range(HPC):
                        nc.tensor.matmul(
                            sc[:, h, D:D + TCH], lhsT=tri_sb, rhs=id_sb,
                            start=False, stop=True, skip_group_check=True,
                        )
                return sc, D

            # ---- prologue: q/k(0) chunk-major (each matmul starts as soon
            # as its xpe chunk DMA lands), first scores, then q/k(1) ----
            def qk_chunk_major(w):
                qs = smp.tile([128, SW], f32, tag="sm", name=f"qk{w}_0")
                ks_ = smp.tile([128, SW], f32, tag="sm", name=f"qk{w}_1")
                qk_store[(w, 0)] = qs
                qk_store[(w, 1)] = ks_
                for k in range(4):
                    for blk, ps in ((0, qs), (1, ks_)):
                        nc.tensor.matmul(
                            ps,
                            lhsT=w3t_sb[:, k, blk * 128:(blk + 1) * 128],
                            rhs=xws[w][:, k, :],
                            start=(k == 0), stop=(k == 3),
                        )
                nc.vector.tensor_scalar_add(
                    out=qTs[w], in0=qs, scalar1=b3_sb[:, 0:1])
                nc.vector.tensor_scalar_add(
                    out=kTs[w], in0=ks_, scalar1=b3_sb[:, 1:2])

            # PE warm-up: ~16 junk matmuls on zeroed tiles fill the initial
            # DMA-latency window and bring HAM out of the half-clock state
            # before real work arrives
            wu_l = singles.tile([128, 128], bf16, tag="wu_l")
            nc.vector.memset(wu_l, 0.0)
            wu_r = singles.tile([128, SW], bf16, tag="wu_r")
            nc.vector.memset(wu_r, 0.0)
            wu_ps = smp.tile([128, SW], f32, tag="sm", name="wu_ps")
            for _ in range(16):
                nc.tensor.matmul(wu_ps, lhsT=wu_l, rhs=wu_r,
                                 start=True, stop=True, skip_group_check=True)

            from collections import deque
            qk_chunk_major(0)
            sc_q = deque([emit_sc(0), emit_sc(1)])
            qk_chunk_major(1)
            for c in range(NCH):
                i, j = chunks[c]
                sc, D = sc_q.popleft()
                pb = pbp.tile([128, HPC, SW], bf16, tag="pb", name=f"pb{c}")
                nc.scalar.activation(
                    out=pb[:, :, D:SW], in_=sc[:, :, D:SW], func=Exp, scale=TEMP,
                )
                for work in fillers.get(c, ()):
                    work()
                if c + 2 < NCH:
                    sc_q.append(emit_sc(c + 2))
                jmax = 4 * i + 3
                wj, jj = j // 4, j % 4
                for h in range(HPC):
                    nc.tensor.matmul(
                        avs_store[i][h][:, D:SW], lhsT=vsb[wj][:, jj, h, :],
                        rhs=pb[:, h, D:SW],
                        start=(j == 0), stop=(j == jmax),
                    )

            # ---- epilogue: window 3 tail + out-projection ----
            tail(NW - 1)
            outproj(NW - 1, (0, 2), ob_engine="scalar")
            outproj(NW - 1, (1, 3))

    nc.compile()
    return nc


def _get_nc():
    if "nc" not in _CACHE:
        _CACHE["nc"] = _build_bass()
    return _CACHE["nc"]


def _make_in_maps(x, pe, Wqkv, bqkv, Wc):
    bf = ml_dtypes.bfloat16
    tt = np.arange(128)[:, None]
    kk = np.arange(128)[None, :]
    tri = np.where(kk < tt, np.float32(BIGNEG), np.float32(0.0)).astype(bf).T.copy()
    id128 = np.eye(128, dtype=np.float32).astype(bf)

    xpet_b = {}
    for b in range(B):
        t = (x[:, b, :] + pe[:, b, :]).T.astype(bf).reshape(4, 128, NW, SW)
        xpet_b[b] = np.ascontiguousarray(t.transpose(2, 0, 1, 3))

    in_maps = []
    for core in range(NCORE):
        b, hg = core // 4, core % 4
        lo = hg * 128
        W3 = np.concatenate([Wqkv[lo:lo + 128], Wqkv[C + lo:C + lo + 128],
                             Wqkv[2 * C + lo:2 * C + lo + 128]])
        w3t = np.ascontiguousarray(W3.T).reshape(4, 128, 384).astype(bf)
        b3 = np.stack([bqkv[lo:lo + 128], bqkv[C + lo:C + lo + 128]], axis=1)
        b3 = np.ascontiguousarray(b3).astype(np.float32)
        wct = np.ascontiguousarray(Wc[:, lo:lo + 128].T).astype(bf)
        in_maps.append({
            "xpet": xpet_b[b], "w3t": w3t, "b3": b3,
            "wct": wct, "tri": tri, "id128": id128,
        })
    return in_maps


def _numpy_fallback(x, pe, content_mask, Wqkv, bqkv, Wc, bc):
    xpe = (x + pe).astype(np.float32)
    qkv = xpe.reshape(-1, C) @ Wqkv.T + bqkv
    qkv = qkv.reshape(S, B, 3 * C)
    q, k, v = np.split(qkv, 3, axis=-1)
    q = q.reshape(S, B, H, CC)
    k = k.reshape(S, B, H, CC)
    v = v.reshape(S, B, H, CC)
    out = np.empty((S, B, C), np.float32)
    for b in range(B):
        for h in range(H):
            sc = (q[:, b, h] @ k[:, b, h].T) * np.float32(TEMP)
            sc = np.where(content_mask[:, :, b], -np.inf, sc)
            sc = sc - sc.max(axis=1, keepdims=True)
            p = np.exp(sc)
            p /= p.sum(axis=1, keepdims=True)
            out[:, b, h * CC:(h + 1) * CC] = p @ v[:, b, h]
    return (out.reshape(-1, C) @ Wc.T + bc).reshape(S, B, C).astype(np.float32)


def kernel(x, pe, content_mask, pad, Wqkv, bqkv, Wc, bc):
    x = np.asarray(x, dtype=np.float32)
    pe = np.asarray(pe, dtype=np.float32)
    content_mask = np.asarray(content_mask)
    Wqkv = np.asarray(Wqkv, dtype=np.float32)
    bqkv = np.asarray(bqkv, dtype=np.float32)
    Wc = np.asarray(Wc, dtype=np.float32)
    bc = np.asarray(bc, dtype=np.float32)

    idx = np.arange(S)
    causal = idx[None, :] > idx[:, None]
    if not np.array_equal(content_mask, np.broadcast_to(causal[:, :, None], (S, S, B))):
        return _numpy_fallback(x, pe, content_mask, Wqkv, bqkv, Wc, bc)

    from concourse.bass_utils import run_bass_kernel_spmd

    nc = _get_nc()
    in_maps = _make_in_maps(x, pe, Wqkv, bqkv, Wc)
    res = run_bass_kernel_spmd(nc, in_maps, core_ids=list(range(NCORE)))
    out = np.empty((S, B, C), np.float32)
    bc_eff = bc + Wc @ bqkv[2 * C:3 * C]   # v-bias folded through the output proj
    for b in range(B):
        acc = res.results[b * 4]["outp"].astype(np.float32).copy()
        for g in range(1, 4):
            acc += res.results[b * 4 + g]["outp"]
        out[:, b, :] = acc.T + bc_eff
    return out
